# Initial kernel scaffold
#
"""DiffJPEG Trainium2 Bass kernel (self-contained).

Strategy: pure data-parallel over 8 NeuronCores (4 images each). Per image the
pipeline is four "data-stationary" matmul stages — the image data rides the PE
stationary operand (lhsT) while small constant block-diagonal DCT matrices
stream as rhs. Each stage contracts the partition dim and transposes the
layout, so after 4 stages the layout returns to natural [rows, cols]:

  S1 [row,col] -> [col,(I,u)] : vertical DCT (+RGB->YCC fold, chroma 2x1 avg)
  S2 -> [(I,u),(J,v)]         : horizontal DCT (+1/fq fold; chroma 1x2 avg)
  quant (DVE/ACT): q=c/T; diff-round via f32 magic-number trick; dequant r*T
  S3 -> [(J,v),(I,x)]         : vertical iDCT (+fq fold; chroma 2x upsample)
  S4 -> [(I,x),col]           : horizontal iDCT (+YCC->RGB, /255, chroma up)

+/-128 DC offsets are folded into the Y DC coefficients (per-image immediates
passed as [128,1] per-partition scalar tensors).
"""
import sys
import numpy as np

sys.path.insert(0, "/opt/trn_rl_repo")

N_CORES = 8
IMGS = 4          # images per core
H = W = 512
MAGIC = 12582912.0  # 1.5*2**23: (x+M)-M == round-half-even(x) for |x|<2**22

# ---------------------------------------------------------------------------
# host-side constants
# ---------------------------------------------------------------------------
_xs = np.arange(8, dtype=np.float32)
_COS = np.cos((2 * _xs[:, None] + 1) * _xs[None, :] * np.pi / 16).astype(np.float32)
_alpha = np.array([1.0 / np.sqrt(2)] + [1.0] * 7, dtype=np.float32)
_Y_TABLE = np.array([
    [16, 11, 10, 16, 24, 40, 51, 61], [12, 12, 14, 19, 26, 58, 60, 55],
    [14, 13, 16, 24, 40, 57, 69, 56], [14, 17, 22, 29, 51, 87, 80, 62],
    [18, 22, 37, 56, 68, 109, 103, 77], [24, 35, 55, 64, 81, 104, 113, 92],
    [49, 64, 78, 87, 103, 121, 120, 101], [72, 92, 95, 98, 112, 100, 103, 99]],
    dtype=np.float32)
_C_TABLE = np.full((8, 8), 99.0, dtype=np.float32)
_C_TABLE[:4, :4] = np.array([[17, 18, 24, 47], [18, 21, 26, 66],
                             [24, 26, 56, 99], [47, 66, 99, 99]], dtype=np.float32)
_RGB2YCC = np.array([[0.299, 0.587, 0.114],
                    [-0.168736, -0.331264, 0.5],
                    [0.5, -0.418688, -0.081312]], dtype=np.float32)
_YCC2RGB = np.array([[1.0, 0.0, 1.402],
                    [1.0, -0.344136, -0.714136],
                    [1.0, 1.772, 0.0]], dtype=np.float32)


def _bd(M, n):
    r, c = M.shape
    out = np.zeros((r * n, c * n), dtype=np.float64)
    for i in range(n):
        out[i * r:(i + 1) * r, i * c:(i + 1) * c] = M
    return out


def _base_mats():
    Av = (_COS.astype(np.float64) * 0.5 * _alpha.astype(np.float64)[None, :])  # [x,u]
    Avi = Av.T.copy()                                   # [u,x]
    Avs = np.zeros((16, 8))                             # subsample fwd
    for x2 in range(16):
        Avs[x2] = Av[x2 // 2] / 2.0
    Avu = np.zeros((8, 16))                             # upsample inv
    for x2 in range(16):
        Avu[:, x2] = Avi[:, x2 // 2]
    return Av, Avi, Avs, Avu


def build_core_inputs(x_core, quality_core):
    """x_core [IMGS,3,512,512] f32, quality_core [IMGS] f32 -> in_map dict."""
    Av, Avi, Avs, Avu = _base_mats()
    f32 = np.float32
    bd16v = _bd(Av, 16)        # [128,128] vertical fwd
    bd8s = _bd(Avs, 8)         # [128,64]  vertical fwd subsampled
    bd16i = _bd(Avi, 16)       # [128,128] vertical inv (also horizontal inv)
    bd16u = _bd(Avu, 16)       # [128,256] vertical inv upsampling
    bd8u = _bd(Avu, 8)         # [64,128]  horizontal inv upsampling

    w1y = np.stack([(255.0 * _RGB2YCC[0, p] * bd16v).astype(f32) for p in range(3)])
    # chroma S1 rhs: cols 0:64 cb, 64:128 cr
    w1c = np.stack([
        np.concatenate([255.0 * _RGB2YCC[1, p] * bd8s,
                        255.0 * _RGB2YCC[2, p] * bd8s], axis=1).astype(f32)
        for p in range(3)])

    fqs = []
    for q in np.asarray(quality_core, dtype=np.float64):
        factor = (5000.0 / q if q < 50.0 else 200.0 - 2.0 * q) / 100.0
        fqs.append(factor)

    w2y = np.stack([(bd16v / fq).astype(f32) for fq in fqs])
    w2c = np.stack([(_bd(Avs, 8) / fq).astype(f32) for fq in fqs])     # [4,128,64]
    w3y = np.stack([(bd16i * fq).astype(f32) for fq in fqs])
    w3c = np.stack([(bd16u * fq).astype(f32) for fq in fqs])           # [4,128,256]
    w4y = (bd16i / 255.0).astype(f32)                                  # [128,128]
    w4c = np.stack([
        (_YCC2RGB[0, 2] / 255.0 * bd8u).astype(f32),   # R <- cr
        (_YCC2RGB[1, 1] / 255.0 * bd8u).astype(f32),   # G <- cb
        (_YCC2RGB[1, 2] / 255.0 * bd8u).astype(f32),   # G <- cr
        (_YCC2RGB[2, 1] / 255.0 * bd8u).astype(f32),   # B <- cb
    ])                                                                 # [4,64,128]

    rho_y = np.tile(1.0 / _Y_TABLE, (16, 1)).astype(f32)               # [128,8]
    t_y = np.tile(_Y_TABLE, (16, 1)).astype(f32)
    rho_c = np.tile(1.0 / _C_TABLE, (16, 1)).astype(f32)
    t_c = np.tile(_C_TABLE, (16, 1)).astype(f32)

    T00 = float(_Y_TABLE[0, 0])
    dcf = np.stack([np.full((128, 1), -1024.0 / (T00 * fq), dtype=f32) for fq in fqs])
    dci = np.stack([np.full((128, 1), 1024.0 / (T00 * fq), dtype=f32) for fq in fqs])

    return {
        "x": np.ascontiguousarray(x_core, dtype=f32),
        "w1y": w1y, "w1c": w1c, "w2y": w2y, "w2c": w2c,
        "w3y": w3y, "w3c": w3c, "w4y": w4y, "w4c": w4c,
        "rho_y": rho_y, "t_y": t_y, "rho_c": rho_c, "t_c": t_c,
        "dcf": dcf, "dci": dci,
    }


# ---------------------------------------------------------------------------
# bass program
# ---------------------------------------------------------------------------
def build_program():
    import concourse.bass as bass
    import concourse.mybir as mybir
    from concourse.tile import TileContext

    dt = mybir.dt.float32
    op = mybir.AluOpType

    nc = bass.Bass("TRN2", target_bir_lowering=False, debug=False,
                   enable_asserts=False, num_devices=N_CORES)

    x_d = nc.dram_tensor("x", [IMGS, 3, H, W], dt, kind="ExternalInput").ap()
    out_d = nc.dram_tensor("out", [IMGS, 3, H, W], dt, kind="ExternalOutput").ap()
    w1y_d = nc.dram_tensor("w1y", [3, 128, 128], dt, kind="ExternalInput").ap()
    w1c_d = nc.dram_tensor("w1c", [3, 128, 128], dt, kind="ExternalInput").ap()
    w2y_d = nc.dram_tensor("w2y", [IMGS, 128, 128], dt, kind="ExternalInput").ap()
    w2c_d = nc.dram_tensor("w2c", [IMGS, 128, 64], dt, kind="ExternalInput").ap()
    w3y_d = nc.dram_tensor("w3y", [IMGS, 128, 128], dt, kind="ExternalInput").ap()
    w3c_d = nc.dram_tensor("w3c", [IMGS, 128, 256], dt, kind="ExternalInput").ap()
    w4y_d = nc.dram_tensor("w4y", [128, 128], dt, kind="ExternalInput").ap()
    w4c_d = nc.dram_tensor("w4c", [4, 64, 128], dt, kind="ExternalInput").ap()
    rho_y_d = nc.dram_tensor("rho_y", [128, 8], dt, kind="ExternalInput").ap()
    t_y_d = nc.dram_tensor("t_y", [128, 8], dt, kind="ExternalInput").ap()
    rho_c_d = nc.dram_tensor("rho_c", [128, 8], dt, kind="ExternalInput").ap()
    t_c_d = nc.dram_tensor("t_c", [128, 8], dt, kind="ExternalInput").ap()
    dcf_d = nc.dram_tensor("dcf", [IMGS, 128, 1], dt, kind="ExternalInput").ap()
    dci_d = nc.dram_tensor("dci", [IMGS, 128, 1], dt, kind="ExternalInput").ap()

    with TileContext(nc, trace_sim=False) as tc:
        with tc.tile_pool(name="consts", bufs=1) as cp, \
             tc.tile_pool(name="pix", bufs=14) as pixp, \
             tc.tile_pool(name="h1", bufs=10) as h1p, \
             tc.tile_pool(name="qq", bufs=7) as qp, \
             tc.tile_pool(name="tmp", bufs=8) as tp, \
             tc.tile_pool(name="r2", bufs=8) as r2p, \
             tc.tile_pool(name="zz", bufs=10) as zp, \
             tc.tile_pool(name="outp", bufs=6) as op_, \
             tc.tile_pool(name="ps", bufs=8, space="PSUM") as pp:

            def cload(ap_dram, shape, tag):
                t = cp.tile(shape, dt, tag=tag)
                nc.sync.dma_start(out=t[:], in_=ap_dram)
                return t

            w1y_s = [cload(w1y_d[p], [128, 128], f"w1y{p}") for p in range(3)]
            w1c_s = [cload(w1c_d[p], [128, 128], f"w1c{p}") for p in range(3)]
            w2y_s = [cload(w2y_d[m], [128, 128], f"w2y{m}") for m in range(IMGS)]
            w2c_s = [cload(w2c_d[m], [128, 64], f"w2c{m}") for m in range(IMGS)]
            w3y_s = [cload(w3y_d[m], [128, 128], f"w3y{m}") for m in range(IMGS)]
            w3c_s = [cload(w3c_d[m], [128, 256], f"w3c{m}") for m in range(IMGS)]
            w4y_s = cload(w4y_d, [128, 128], "w4y")
            w4c_s = [cload(w4c_d[k], [64, 128], f"w4c{k}") for k in range(4)]
            rho_y_s = cload(rho_y_d, [128, 8], "rho_y")
            t_y_s = cload(t_y_d, [128, 8], "t_y")
            rho_c_s = cload(rho_c_d, [128, 8], "rho_c")
            t_c_s = cload(t_c_d, [128, 8], "t_c")
            dcf_s = [cload(dcf_d[m], [128, 1], f"dcf{m}") for m in range(IMGS)]
            dci_s = [cload(dci_d[m], [128, 1], f"dci{m}") for m in range(IMGS)]

            def bcast8(t):  # [128,8] const -> [128,64,8] step-0 broadcast (==512)
                return t[:, None, :].broadcast_to((128, 64, 8))

            def quant(ps_tile, rho_s, t_s, m):
                """psum coeff tile [128,512] -> dequantized r2 sbuf tile.
                m >= 0: Y-image index (apply DC adjusts); m < 0: chroma."""
                q = qp.tile([128, 512], dt, tag="q")
                s = tp.tile([128, 512], dt, tag="s")
                dp_ = tp.tile([128, 512], dt, tag="dp")
                d2 = tp.tile([128, 512], dt, tag="d2")
                g = tp.tile([128, 512], dt, tag="g")
                r2t = r2p.tile([128, 512], dt, tag="r2")
                # q = coeff * (1/T)   (reads PSUM, writes SBUF)
                nc.vector.tensor_tensor(out=q[:], in0=ps_tile[:], in1=bcast8(rho_s),
                                        op=op.mult)
                if m >= 0:
                    nc.vector.tensor_scalar(
                        out=q[0:128:8, 0:512:8], in0=q[0:128:8, 0:512:8],
                        scalar1=dcf_s[m][0:128:8, :], scalar2=None, op0=op.add)
                # s = round(q) via magic trick
                nc.vector.tensor_scalar(out=s[:], in0=q[:], scalar1=MAGIC,
                                        scalar2=MAGIC, op0=op.add, op1=op.subtract)
                # dp = round(q) - q = -d
                nc.vector.tensor_tensor(out=dp_[:], in0=s[:], in1=q[:],
                                        op=op.subtract)
                nc.scalar.square(out=d2[:], in_=dp_[:])
                # g = (d2-1)*dp = d - d^3
                nc.vector.scalar_tensor_tensor(out=g[:], in0=d2[:], scalar=1.0,
                                               in1=dp_[:], op0=op.subtract,
                                               op1=op.mult)
                # r = q - g = round(q) + d^3
                nc.vector.tensor_tensor(out=s[:], in0=q[:], in1=g[:],
                                        op=op.subtract)
                if m >= 0:
                    nc.vector.tensor_scalar(
                        out=s[0:128:8, 0:512:8], in0=s[0:128:8, 0:512:8],
                        scalar1=dci_s[m][0:128:8, :], scalar2=None, op0=op.add)
                # r2 = r * T
                nc.vector.tensor_tensor(out=r2t[:], in0=s[:], in1=bcast8(t_s),
                                        op=op.mult)
                return r2t

            for m in range(IMGS):
                # ---- load pixel planes ----
                pix = [[pixp.tile([128, 512], dt, tag="pix") for _ in range(4)]
                       for _ in range(3)]
                for p in range(3):
                    for i in range(4):
                        nc.sync.dma_start(out=pix[p][i][:],
                                          in_=x_d[m, p, 128 * i:128 * (i + 1), :])

                # ---- S1: vertical DCT (+color fold) ----
                h1y = []
                h1c = []
                for j in range(4):
                    psY = pp.tile([128, 512], dt, tag="ps")
                    psC = pp.tile([128, 512], dt, tag="ps")
                    for i in range(4):
                        for p in range(3):
                            lhs = pix[p][i][:, 128 * j:128 * (j + 1)]
                            nc.tensor.matmul(psY[:, 128 * i:128 * (i + 1)],
                                             lhsT=lhs, rhs=w1y_s[p][:],
                                             start=(p == 0), stop=(p == 2))
                            nc.tensor.matmul(psC[:, 128 * i:128 * (i + 1)],
                                             lhsT=lhs, rhs=w1c_s[p][:],
                                             start=(p == 0), stop=(p == 2))
                    ty = h1p.tile([128, 512], dt, tag="h1y")
                    nc.scalar.copy(out=ty[:], in_=psY[:])
                    h1y.append(ty)
                    tch = h1p.tile([128, 512], dt, tag="h1c")
                    # reorder [i,(cb|cr),64] -> [(cb|cr),i,64]
                    nc.scalar.copy(out=tch[:],
                                   in_=psC[:].rearrange("p (i c v) -> p (c i v)",
                                                        i=4, c=2, v=64))
                    h1c.append(tch)

                # ---- S2 + quant: Y ----
                r2y = []
                for i in range(4):
                    psQ = pp.tile([128, 512], dt, tag="ps")
                    for j in range(4):
                        nc.tensor.matmul(psQ[:, 128 * j:128 * (j + 1)],
                                         lhsT=h1y[j][:, 128 * i:128 * (i + 1)],
                                         rhs=w2y_s[m][:], start=True, stop=True)
                    r2y.append(quant(psQ, rho_y_s, t_y_s, m))

                # ---- S2 + quant: chroma ----
                r2c = []
                for ch in range(2):
                    psQ = pp.tile([128, 512], dt, tag="ps")
                    for k in range(2):
                        for j in range(4):
                            lhs = h1c[j][:, 256 * ch + 128 * k:256 * ch + 128 * (k + 1)]
                            nc.tensor.matmul(
                                psQ[:, 256 * k + 64 * j:256 * k + 64 * (j + 1)],
                                lhsT=lhs, rhs=w2c_s[m][:], start=True, stop=True)
                    r2c.append(quant(psQ, rho_c_s, t_c_s, -1))

                # ---- S3: vertical iDCT ----
                zy = []
                for j in range(4):
                    psZ = pp.tile([128, 512], dt, tag="ps")
                    for i in range(4):
                        nc.tensor.matmul(psZ[:, 128 * i:128 * (i + 1)],
                                         lhsT=r2y[i][:, 128 * j:128 * (j + 1)],
                                         rhs=w3y_s[m][:], start=True, stop=True)
                    t_ = zp.tile([128, 512], dt, tag="zy")
                    nc.scalar.copy(out=t_[:], in_=psZ[:])
                    zy.append(t_)
                zc = [[], []]
                for ch in range(2):
                    for t in range(2):
                        psZ = pp.tile([128, 512], dt, tag="ps")
                        for k in range(2):
                            lhs = r2c[ch][:, 256 * k + 128 * t:256 * k + 128 * (t + 1)]
                            nc.tensor.matmul(psZ[:, 256 * k:256 * (k + 1)],
                                             lhsT=lhs, rhs=w3c_s[m][:],
                                             start=True, stop=True)
                        t_ = zp.tile([128, 512], dt, tag="zc")
                        nc.scalar.copy(out=t_[:], in_=psZ[:])
                        zc[ch].append(t_)

                # ---- S4: horizontal iDCT + color + clamp + store ----
                for i in range(4):
                    psO = [pp.tile([128, 512], dt, tag="ps") for _ in range(3)]
                    for j in range(4):
                        ldY = zy[j][:, 128 * i:128 * (i + 1)]
                        for pl in range(3):
                            nc.tensor.matmul(psO[pl][:, 128 * j:128 * (j + 1)],
                                             lhsT=ldY, rhs=w4y_s[:],
                                             start=True, stop=False)
                        cb_l = zc[0][j // 2][64 * (j % 2):64 * (j % 2) + 64,
                                             128 * i:128 * (i + 1)]
                        cr_l = zc[1][j // 2][64 * (j % 2):64 * (j % 2) + 64,
                                             128 * i:128 * (i + 1)]
                        sl = slice(128 * j, 128 * (j + 1))
                        nc.tensor.matmul(psO[0][:, sl], lhsT=cr_l, rhs=w4c_s[0][:],
                                         start=False, stop=True)
                        nc.tensor.matmul(psO[1][:, sl], lhsT=cb_l, rhs=w4c_s[1][:],
                                         start=False, stop=False)
                        nc.tensor.matmul(psO[1][:, sl], lhsT=cr_l, rhs=w4c_s[2][:],
                                         start=False, stop=True)
                        nc.tensor.matmul(psO[2][:, sl], lhsT=cb_l, rhs=w4c_s[3][:],
                                         start=False, stop=True)
                    for pl in range(3):
                        o = op_.tile([128, 512], dt, tag="o")
                        nc.vector.tensor_scalar(out=o[:], in0=psO[pl][:],
                                                scalar1=0.0, scalar2=1.0,
                                                op0=op.max, op1=op.min)
                        nc.sync.dma_start(
                            out=out_d[m, pl, 128 * i:128 * (i + 1), :], in_=o[:])
    return nc


_NC_CACHE = {}


def _get_nc():
    if "nc" not in _NC_CACHE:
        _NC_CACHE["nc"] = build_program()
    return _NC_CACHE["nc"]


def kernel(x, quality):
    """Full inputs -> full output. Shards batch over 8 cores internally."""
    from concourse import bass_utils
    x = np.asarray(x, dtype=np.float32)
    quality = np.asarray(quality, dtype=np.float32)
    B = x.shape[0]
    assert B == N_CORES * IMGS, (B, N_CORES, IMGS)
    nc = _get_nc()
    in_maps = []
    for c in range(N_CORES):
        sl = slice(c * IMGS, (c + 1) * IMGS)
        in_maps.append(build_core_inputs(x[sl], quality[sl]))
    res = bass_utils.run_bass_kernel_spmd(nc, in_maps, core_ids=list(range(N_CORES)))
    outs = [res.results[c]["out"] for c in range(N_CORES)]
    return np.concatenate(outs, axis=0).astype(np.float32)


# revision 9
# speedup vs baseline: 1.0347x; 1.0347x over previous
"""DiffJPEG Trainium2 Bass kernel (self-contained).

Strategy: pure data-parallel over 8 NeuronCores (4 images each). Per image the
pipeline is four "data-stationary" matmul stages — the image data rides the PE
stationary operand (lhsT) while small constant block-diagonal DCT matrices
stream as rhs. Each stage contracts the partition dim and transposes the
layout, so after 4 stages the layout returns to natural [rows, cols]:

  S1 [row,col] -> [col,(I,u)] : vertical DCT (+RGB->YCC fold, chroma 2x1 avg)
  S2 -> [(I,u),(J,v)]         : horizontal DCT (+1/fq fold; chroma 1x2 avg)
  quant (DVE/ACT): q=c/T; diff-round via f32 magic-number trick; dequant r*T
  S3 -> [(J,v),(I,x)]         : vertical iDCT (+fq fold; chroma 2x upsample)
  S4 -> [(I,x),col]           : horizontal iDCT (+YCC->RGB, /255, chroma up)

+/-128 DC offsets are folded into the Y DC coefficients (per-image immediates
passed as [128,1] per-partition scalar tensors).
"""
import sys
import numpy as np

sys.path.insert(0, "/opt/trn_rl_repo")

N_CORES = 8
IMGS = 4          # images per core
H = W = 512
MAGIC = 12582912.0  # 1.5*2**23: (x+M)-M == round-half-even(x) for |x|<2**22

# ---------------------------------------------------------------------------
# host-side constants
# ---------------------------------------------------------------------------
_xs = np.arange(8, dtype=np.float32)
_COS = np.cos((2 * _xs[:, None] + 1) * _xs[None, :] * np.pi / 16).astype(np.float32)
_alpha = np.array([1.0 / np.sqrt(2)] + [1.0] * 7, dtype=np.float32)
_Y_TABLE = np.array([
    [16, 11, 10, 16, 24, 40, 51, 61], [12, 12, 14, 19, 26, 58, 60, 55],
    [14, 13, 16, 24, 40, 57, 69, 56], [14, 17, 22, 29, 51, 87, 80, 62],
    [18, 22, 37, 56, 68, 109, 103, 77], [24, 35, 55, 64, 81, 104, 113, 92],
    [49, 64, 78, 87, 103, 121, 120, 101], [72, 92, 95, 98, 112, 100, 103, 99]],
    dtype=np.float32)
_C_TABLE = np.full((8, 8), 99.0, dtype=np.float32)
_C_TABLE[:4, :4] = np.array([[17, 18, 24, 47], [18, 21, 26, 66],
                             [24, 26, 56, 99], [47, 66, 99, 99]], dtype=np.float32)
_RGB2YCC = np.array([[0.299, 0.587, 0.114],
                    [-0.168736, -0.331264, 0.5],
                    [0.5, -0.418688, -0.081312]], dtype=np.float32)
_YCC2RGB = np.array([[1.0, 0.0, 1.402],
                    [1.0, -0.344136, -0.714136],
                    [1.0, 1.772, 0.0]], dtype=np.float32)


def _bd(M, n):
    r, c = M.shape
    out = np.zeros((r * n, c * n), dtype=np.float64)
    for i in range(n):
        out[i * r:(i + 1) * r, i * c:(i + 1) * c] = M
    return out


def _base_mats():
    Av = (_COS.astype(np.float64) * 0.5 * _alpha.astype(np.float64)[None, :])  # [x,u]
    Avi = Av.T.copy()                                   # [u,x]
    Avs = np.zeros((16, 8))                             # subsample fwd
    for x2 in range(16):
        Avs[x2] = Av[x2 // 2] / 2.0
    Avu = np.zeros((8, 16))                             # upsample inv
    for x2 in range(16):
        Avu[:, x2] = Avi[:, x2 // 2]
    return Av, Avi, Avs, Avu


def build_core_inputs(x_core, quality_core):
    """x_core [IMGS,3,512,512] f32, quality_core [IMGS] f32 -> in_map dict."""
    Av, Avi, Avs, Avu = _base_mats()
    f32 = np.float32
    bd16v = _bd(Av, 16)        # [128,128] vertical fwd
    bd8s = _bd(Avs, 8)         # [128,64]  vertical fwd subsampled
    bd16i = _bd(Avi, 16)       # [128,128] vertical inv (also horizontal inv)
    bd16u = _bd(Avu, 16)       # [128,256] vertical inv upsampling
    bd8u = _bd(Avu, 8)         # [64,128]  horizontal inv upsampling

    w1y = np.stack([(255.0 * _RGB2YCC[0, p] * bd16v).astype(f32) for p in range(3)])
    # chroma S1 rhs: cols 0:64 cb, 64:128 cr
    w1c = np.stack([
        np.concatenate([255.0 * _RGB2YCC[1, p] * bd8s,
                        255.0 * _RGB2YCC[2, p] * bd8s], axis=1).astype(f32)
        for p in range(3)])

    fqs = []
    for q in np.asarray(quality_core, dtype=np.float64):
        factor = (5000.0 / q if q < 50.0 else 200.0 - 2.0 * q) / 100.0
        fqs.append(factor)

    w2y = np.stack([(bd16v / fq).astype(f32) for fq in fqs])
    w2c = np.stack([(_bd(Avs, 8) / fq).astype(f32) for fq in fqs])     # [4,128,64]
    w3y = np.stack([(bd16i * fq).astype(f32) for fq in fqs])
    w3c = np.stack([(bd16u * fq).astype(f32) for fq in fqs])           # [4,128,256]
    w4y = (bd16i / 255.0).astype(f32)                                  # [128,128]
    # zero-padded to K=128 so lhsT can be a full 128-partition tile:
    # parity 0 -> wanted J'-blocks in partitions 0:64, parity 1 -> 64:128
    z64 = np.zeros((64, 128))
    combos = [
        _YCC2RGB[0, 2] / 255.0 * bd8u,   # R <- cr
        _YCC2RGB[1, 1] / 255.0 * bd8u,   # G <- cb
        _YCC2RGB[1, 2] / 255.0 * bd8u,   # G <- cr
        _YCC2RGB[2, 1] / 255.0 * bd8u,   # B <- cb
    ]
    w4c = np.stack([
        np.stack([np.concatenate([c_, z64], axis=0),
                  np.concatenate([z64, c_], axis=0)]).astype(f32)
        for c_ in combos])                                             # [4,2,128,128]

    rho_y = np.tile(1.0 / _Y_TABLE, (16, 1)).astype(f32)               # [128,8]
    t_y = np.tile(_Y_TABLE, (16, 1)).astype(f32)
    rho_c = np.tile(1.0 / _C_TABLE, (16, 1)).astype(f32)
    t_c = np.tile(_C_TABLE, (16, 1)).astype(f32)

    T00 = float(_Y_TABLE[0, 0])
    # nonzero only on u==0 partitions (p%8==0): the DC-adjust op runs on all
    # 128 partitions (stride-1, HW requirement) but only v==0 free columns.
    mask = (np.arange(128) % 8 == 0).astype(f32)[:, None]
    dcf = np.stack([(mask * (-1024.0 / (T00 * fq))).astype(f32) for fq in fqs])
    dci = np.stack([(mask * (1024.0 / (T00 * fq))).astype(f32) for fq in fqs])

    return {
        "x": np.ascontiguousarray(x_core, dtype=f32),
        "w1y": w1y, "w1c": w1c, "w2y": w2y, "w2c": w2c,
        "w3y": w3y, "w3c": w3c, "w4y": w4y, "w4c": w4c,
        "rho_y": rho_y, "t_y": t_y, "rho_c": rho_c, "t_c": t_c,
        "dcf": dcf, "dci": dci,
    }


# ---------------------------------------------------------------------------
# bass program
# ---------------------------------------------------------------------------
def build_program(repeat=1):
    import concourse.bacc as bacc
    import concourse.mybir as mybir
    from concourse.tile import TileContext

    dt = mybir.dt.float32
    op = mybir.AluOpType

    nc = bacc.Bacc("TRN2", target_bir_lowering=False, debug=False,
                   enable_asserts=False, num_devices=N_CORES)

    x_d = nc.dram_tensor("x", [IMGS, 3, H, W], dt, kind="ExternalInput").ap()
    out_d = nc.dram_tensor("out", [IMGS, 3, H, W], dt, kind="ExternalOutput").ap()
    w1y_d = nc.dram_tensor("w1y", [3, 128, 128], dt, kind="ExternalInput").ap()
    w1c_d = nc.dram_tensor("w1c", [3, 128, 128], dt, kind="ExternalInput").ap()
    w2y_d = nc.dram_tensor("w2y", [IMGS, 128, 128], dt, kind="ExternalInput").ap()
    w2c_d = nc.dram_tensor("w2c", [IMGS, 128, 64], dt, kind="ExternalInput").ap()
    w3y_d = nc.dram_tensor("w3y", [IMGS, 128, 128], dt, kind="ExternalInput").ap()
    w3c_d = nc.dram_tensor("w3c", [IMGS, 128, 256], dt, kind="ExternalInput").ap()
    w4y_d = nc.dram_tensor("w4y", [128, 128], dt, kind="ExternalInput").ap()
    w4c_d = nc.dram_tensor("w4c", [4, 2, 128, 128], dt, kind="ExternalInput").ap()
    rho_y_d = nc.dram_tensor("rho_y", [128, 8], dt, kind="ExternalInput").ap()
    t_y_d = nc.dram_tensor("t_y", [128, 8], dt, kind="ExternalInput").ap()
    rho_c_d = nc.dram_tensor("rho_c", [128, 8], dt, kind="ExternalInput").ap()
    t_c_d = nc.dram_tensor("t_c", [128, 8], dt, kind="ExternalInput").ap()
    dcf_d = nc.dram_tensor("dcf", [IMGS, 128, 1], dt, kind="ExternalInput").ap()
    dci_d = nc.dram_tensor("dci", [IMGS, 128, 1], dt, kind="ExternalInput").ap()

    with TileContext(nc, trace_sim=False) as tc:
        with tc.tile_pool(name="consts", bufs=1) as cp, \
             tc.tile_pool(name="pix", bufs=14) as pixp, \
             tc.tile_pool(name="h1", bufs=5) as h1p, \
             tc.tile_pool(name="qq", bufs=7) as qp, \
             tc.tile_pool(name="tmp", bufs=2) as tp, \
             tc.tile_pool(name="r2", bufs=8) as r2p, \
             tc.tile_pool(name="zz", bufs=5) as zp, \
             tc.tile_pool(name="outp", bufs=4) as op_, \
             tc.tile_pool(name="ps", bufs=8, space="PSUM") as pp:

            def cload(ap_dram, shape, tag):
                t = cp.tile(shape, dt, tag=tag, name=tag)
                nc.sync.dma_start(out=t[:], in_=ap_dram)
                return t

            w1y_s = [cload(w1y_d[p], [128, 128], f"w1y{p}") for p in range(3)]
            w1c_s = [cload(w1c_d[p], [128, 128], f"w1c{p}") for p in range(3)]
            w2y_s = [cload(w2y_d[m], [128, 128], f"w2y{m}") for m in range(IMGS)]
            w2c_s = [cload(w2c_d[m], [128, 64], f"w2c{m}") for m in range(IMGS)]
            w3y_s = [cload(w3y_d[m], [128, 128], f"w3y{m}") for m in range(IMGS)]
            w3c_s = [cload(w3c_d[m], [128, 256], f"w3c{m}") for m in range(IMGS)]
            w4y_s = cload(w4y_d, [128, 128], "w4y")
            w4c_s = [[cload(w4c_d[k, par], [128, 128], f"w4c{k}_{par}")
                      for par in range(2)] for k in range(4)]
            rho_y_s = cload(rho_y_d, [128, 8], "rho_y")
            t_y_s = cload(t_y_d, [128, 8], "t_y")
            rho_c_s = cload(rho_c_d, [128, 8], "rho_c")
            t_c_s = cload(t_c_d, [128, 8], "t_c")
            dcf_s = [cload(dcf_d[m], [128, 1], f"dcf{m}") for m in range(IMGS)]
            dci_s = [cload(dci_d[m], [128, 1], f"dci{m}") for m in range(IMGS)]

            def bcast8(t):  # [128,8] const -> [128,64,8] step-0 broadcast (==512)
                return t[:, None, :].broadcast_to((128, 64, 8))


            def quant(ps_tile, rho_s, t_s, m):
                """psum coeff tile [128,512] -> dequantized r2 sbuf tile.
                m >= 0: Y-image index (apply DC adjusts); m < 0: chroma."""
                q = qp.tile([128, 512], dt, tag="q", name="q")
                s = tp.tile([128, 512], dt, tag="s", name="s")
                dp_ = tp.tile([128, 512], dt, tag="dp", name="dp")
                d2 = tp.tile([128, 512], dt, tag="d2", name="d2")
                g = tp.tile([128, 512], dt, tag="g", name="g")
                r2t = r2p.tile([128, 512], dt, tag="r2", name="r2")
                # q = coeff * (1/T)   (reads PSUM, writes SBUF)
                nc.vector.tensor_tensor(out=q[:], in0=ps_tile[:], in1=bcast8(rho_s),
                                        op=op.mult)
                if m >= 0:
                    nc.vector.tensor_scalar(
                        out=q[:, 0:512:8], in0=q[:, 0:512:8],
                        scalar1=dcf_s[m][:], scalar2=None, op0=op.add)
                # s = round(q) via magic trick
                nc.vector.tensor_scalar(out=s[:], in0=q[:], scalar1=MAGIC,
                                        scalar2=MAGIC, op0=op.add, op1=op.subtract)
                # dp = round(q) - q = -d
                nc.vector.tensor_tensor(out=dp_[:], in0=s[:], in1=q[:],
                                        op=op.subtract)
                nc.scalar.square(out=d2[:], in_=dp_[:])
                # g = (d2-1)*dp = d - d^3
                nc.vector.scalar_tensor_tensor(out=g[:], in0=d2[:], scalar=1.0,
                                               in1=dp_[:], op0=op.subtract,
                                               op1=op.mult)
                # r = q - g = round(q) + d^3
                nc.vector.tensor_tensor(out=s[:], in0=q[:], in1=g[:],
                                        op=op.subtract)
                if m >= 0:
                    nc.vector.tensor_scalar(
                        out=s[:, 0:512:8], in0=s[:, 0:512:8],
                        scalar1=dci_s[m][:], scalar2=None, op0=op.add)
                # r2 = r * T
                nc.vector.tensor_tensor(out=r2t[:], in0=s[:], in1=bcast8(t_s),
                                        op=op.mult)
                return r2t

            def _build_images():
              for m in range(IMGS):
                # ---- load pixel planes ----
                pix = [[pixp.tile([128, 512], dt, tag="pix", name="pix") for _ in range(4)]
                       for _ in range(3)]
                for p in range(3):
                    for i in range(4):
                        nc.sync.dma_start(out=pix[p][i][:],
                                          in_=x_d[m, p, 128 * i:128 * (i + 1), :])

                # ---- S1: vertical DCT (+color fold) ----
                h1y = []
                h1c = []
                for j in range(4):
                    psY = pp.tile([128, 512], dt, tag="ps", name="psY")
                    psC = pp.tile([128, 512], dt, tag="ps", name="psC")
                    for i in range(4):
                        for p in range(3):
                            lhs = pix[p][i][:, 128 * j:128 * (j + 1)]
                            nc.tensor.matmul(psY[:, 128 * i:128 * (i + 1)],
                                             lhsT=lhs, rhs=w1y_s[p][:],
                                             start=(p == 0), stop=(p == 2))
                            nc.tensor.matmul(psC[:, 128 * i:128 * (i + 1)],
                                             lhsT=lhs, rhs=w1c_s[p][:],
                                             start=(p == 0), stop=(p == 2))
                    ty = h1p.tile([128, 512], dt, tag="h1y", name="h1y")
                    nc.scalar.copy(out=ty[:], in_=psY[:])
                    h1y.append(ty)
                    tch = h1p.tile([128, 512], dt, tag="h1c", name="h1c")
                    # reorder [i,(cb|cr),64] -> [(cb|cr),i,64] via strided out AP
                    nc.scalar.copy(
                        out=tch[:].rearrange("p (c i v) -> p i c v", c=2, i=4, v=64),
                        in_=psC[:].rearrange("p (i c v) -> p i c v", i=4, c=2, v=64))
                    h1c.append(tch)

                # ---- S2 + quant: Y ----
                r2y = []
                for i in range(4):
                    psQ = pp.tile([128, 512], dt, tag="ps", name="psQ")
                    for j in range(4):
                        nc.tensor.matmul(psQ[:, 128 * j:128 * (j + 1)],
                                         lhsT=h1y[j][:, 128 * i:128 * (i + 1)],
                                         rhs=w2y_s[m][:], start=True, stop=True)
                    r2y.append(quant(psQ, rho_y_s, t_y_s, m))

                # ---- S2 + quant: chroma ----
                r2c = []
                for ch in range(2):
                    psQ = pp.tile([128, 512], dt, tag="ps", name="psQ")
                    for k in range(2):
                        for j in range(4):
                            lhs = h1c[j][:, 256 * ch + 128 * k:256 * ch + 128 * (k + 1)]
                            nc.tensor.matmul(
                                psQ[:, 256 * k + 64 * j:256 * k + 64 * (j + 1)],
                                lhsT=lhs, rhs=w2c_s[m][:], start=True, stop=True)
                    r2c.append(quant(psQ, rho_c_s, t_c_s, -1))

                # ---- S3: vertical iDCT ----
                zy = []
                for j in range(4):
                    psZ = pp.tile([128, 512], dt, tag="ps", name="psZ")
                    for i in range(4):
                        nc.tensor.matmul(psZ[:, 128 * i:128 * (i + 1)],
                                         lhsT=r2y[i][:, 128 * j:128 * (j + 1)],
                                         rhs=w3y_s[m][:], start=True, stop=True)
                    t_ = zp.tile([128, 512], dt, tag="zy", name="zy")
                    nc.scalar.copy(out=t_[:], in_=psZ[:])
                    zy.append(t_)
                zc = [[], []]
                for ch in range(2):
                    for t in range(2):
                        psZ = pp.tile([128, 512], dt, tag="ps", name="psZ")
                        for k in range(2):
                            lhs = r2c[ch][:, 256 * k + 128 * t:256 * k + 128 * (t + 1)]
                            nc.tensor.matmul(psZ[:, 256 * k:256 * (k + 1)],
                                             lhsT=lhs, rhs=w3c_s[m][:],
                                             start=True, stop=True)
                        t_ = zp.tile([128, 512], dt, tag="zc", name="zc")
                        nc.scalar.copy(out=t_[:], in_=psZ[:])
                        zc[ch].append(t_)

                # ---- S4: horizontal iDCT + color + clamp + store ----
                for i in range(4):
                    psO = [pp.tile([128, 512], dt, tag="ps", name="psO") for _ in range(3)]
                    for j in range(4):
                        ldY = zy[j][:, 128 * i:128 * (i + 1)]
                        for pl in range(3):
                            nc.tensor.matmul(psO[pl][:, 128 * j:128 * (j + 1)],
                                             lhsT=ldY, rhs=w4y_s[:],
                                             start=True, stop=False)
                        par = j % 2
                        cb_l = zc[0][j // 2][:, 128 * i:128 * (i + 1)]
                        cr_l = zc[1][j // 2][:, 128 * i:128 * (i + 1)]
                        sl = slice(128 * j, 128 * (j + 1))
                        nc.tensor.matmul(psO[0][:, sl], lhsT=cr_l,
                                         rhs=w4c_s[0][par][:],
                                         start=False, stop=True)
                        nc.tensor.matmul(psO[1][:, sl], lhsT=cb_l,
                                         rhs=w4c_s[1][par][:],
                                         start=False, stop=False)
                        nc.tensor.matmul(psO[1][:, sl], lhsT=cr_l,
                                         rhs=w4c_s[2][par][:],
                                         start=False, stop=True)
                        nc.tensor.matmul(psO[2][:, sl], lhsT=cb_l,
                                         rhs=w4c_s[3][par][:],
                                         start=False, stop=True)
                    for pl in range(3):
                        o = op_.tile([128, 512], dt, tag="o", name="o")
                        nc.vector.tensor_scalar(out=o[:], in0=psO[pl][:],
                                                scalar1=0.0, scalar2=1.0,
                                                op0=op.max, op1=op.min)
                        nc.sync.dma_start(
                            out=out_d[m, pl, 128 * i:128 * (i + 1), :], in_=o[:])
            if repeat == 1:
                _build_images()
            else:
                with tc.For_i(0, repeat, 1):
                    _build_images()
    nc.compile()
    return nc


_NC_CACHE = {}


def _get_nc():
    if "nc" not in _NC_CACHE:
        _NC_CACHE["nc"] = build_program()
    return _NC_CACHE["nc"]


def kernel(x, quality):
    """Full inputs -> full output. Shards batch over 8 cores internally."""
    from concourse import bass_utils
    x = np.asarray(x, dtype=np.float32)
    quality = np.asarray(quality, dtype=np.float32)
    B = x.shape[0]
    assert B == N_CORES * IMGS, (B, N_CORES, IMGS)
    nc = _get_nc()
    in_maps = []
    for c in range(N_CORES):
        sl = slice(c * IMGS, (c + 1) * IMGS)
        in_maps.append(build_core_inputs(x[sl], quality[sl]))
    res = bass_utils.run_bass_kernel_spmd(nc, in_maps, core_ids=list(range(N_CORES)))
    outs = [res.results[c]["out"] for c in range(N_CORES)]
    return np.concatenate(outs, axis=0).astype(np.float32)
